# revision 38
# baseline (speedup 1.0000x reference)
"""Causal self-attention (B=2, T=2048, L=1024, H=16) on 8 TRN2 NeuronCores.

Sharding: tensor-parallel over heads (4 heads/core) x data-parallel over batch
(cores 0-3 -> batch 0, cores 4-7 -> batch 1). Each core computes its heads'
attention plus the partial output projection; the host sums the 4 partials
per batch.

Per-core schedule (all matmuls bf16, fp32 PSUM):
  Interleaved by q-block so the scalar engine's exp stream starts early:
    for qb in 0..3:
      sweep(qb):  Q^T/K^T columns [qb*512:(qb+1)*512) accumulated per
                  x-chunk as input DMAs land (qb=0 hides under the loads)
      va(qb):     V tiles for t in [qb*512, (qb+1)*512), ones-augmented
                  per head for the softmax denominator
      B(qb):      S^T chunks [128 k, 2 heads x <=512 q] in a 2-bank PSUM
                  tile -> single merged exp -> PV accumulation; QK of
                  chunk i+1 issues before PV of chunk i so the PE never
                  waits on the exp
      C(qb):      out = y^T.T @ W_proj per t-tile, DMA'd straight from
                  PSUM (no SBUF staging)
  PSUM->SBUF copies run on the otherwise-idle Pool engine; exp owns the
  scalar engine; reciprocal+normalize run on DVE reading PSUM directly.
"""

import sys

for _p in ("/opt/trn_rl_repo",):
    if _p not in sys.path:
        sys.path.insert(0, _p)

import numpy as np

import concourse.bass as bass
import concourse.mybir as mybir
import concourse.tile as tile

F32 = mybir.dt.float32
BF16 = mybir.dt.bfloat16
FP8 = mybir.dt.float8e4
DR = mybir.MatmulPerfMode.DoubleRow
EXP = mybir.ActivationFunctionType.Exp
COPY = mybir.ActivationFunctionType.Copy

B, T, L = 2, 2048, 1024
H = 16
DH = 64                      # head dim
HPC = 4                      # heads per core
HG = HPC * DH                # 256 cols per core per q/k/v
N_CORES = 8
KC = T // 128                # 16 k-chunks
NQB = T // 512               # 4 q-blocks
SCALE = 1.0 / np.sqrt(np.float32(L))  # rsqrt(L) per reference


def build_nc(reps=1):
    nc = bass.Bass("TRN2", target_bir_lowering=False, debug=False)

    xT = nc.dram_tensor("xT", [L, T], BF16, kind="ExternalInput").ap()
    x8d = nc.dram_tensor("x8", [128, 8 * T], FP8, kind="ExternalInput").ap()
    wav = nc.dram_tensor("wav", [L, HG], BF16, kind="ExternalInput").ap()
    wa8d = nc.dram_tensor("wa8", [128, 8 * 2 * HG], FP8, kind="ExternalInput").ap()
    wp = nc.dram_tensor("wp", [HG, L], BF16, kind="ExternalInput").ap()
    tm = nc.dram_tensor("trimaskb", [128, 128], F32, kind="ExternalInput").ap()
    out = nc.dram_tensor("out", [T, L], BF16, kind="ExternalOutput").ap()

    with tile.TileContext(nc) as tc:
        with (
            tc.tile_pool(name="consts", bufs=1) as consts,
            tc.tile_pool(name="xp", bufs=4) as xp,
            tc.tile_pool(name="x8p", bufs=4) as x8p,
            tc.tile_pool(name="qk8w", bufs=1) as qk8w,
            tc.tile_pool(name="wap", bufs=1) as wap,
            tc.tile_pool(name="wpp", bufs=2) as wpp,
            tc.tile_pool(name="qk", bufs=2) as qk,
            tc.tile_pool(name="f8p", bufs=4) as f8p,
            tc.tile_pool(name="vp", bufs=16) as vp,
            tc.tile_pool(name="ytp", bufs=2) as ytp,
            tc.tile_pool(name="ptp", bufs=4) as ptp,
            tc.tile_pool(name="recp", bufs=4) as recp,
            tc.tile_pool(name="bsp", bufs=2) as bsp,
            tc.tile_pool(name="outp", bufs=4) as outp,
            tc.tile_pool(name="pss", bufs=2, space="PSUM") as pss,
            tc.tile_pool(name="pso", bufs=2, space="PSUM") as pso,
            tc.tile_pool(name="psm", bufs=2, space="PSUM") as psm,
        ):
          def _body(_rep):
            _sfx = "" if reps == 1 else f"_r{_rep}"
            # ---- constants & weight loads (gpsimd queue, parallel to sync) --
            tm_sb = consts.tile([128, 128], BF16, tag="tm", name=f"tm_sb{_sfx}")
            nc.gpsimd.dma_start(out=tm_sb[:], in_=tm[:])
            ones_sb = consts.tile([128, 128], BF16, tag="ones", name=f"ones_sb{_sfx}")
            nc.vector.memset(ones_sb[:], 1.0)
            # preload the Exp table so the first real exp skips the swap
            warm = consts.tile([1, 8], BF16, tag="warm", name=f"warm{_sfx}")
            nc.vector.memset(warm[:], 0.0)
            nc.scalar.activation(warm[:], warm[:], EXP)
            wp_sb = []
            for i in range(2):
                wpt = wpp.tile([128, L], BF16, tag="wp", name=f"wpt{i}{_sfx}")
                nc.gpsimd.dma_start(out=wpt[:], in_=wp[i * 128:(i + 1) * 128, :])
                wp_sb.append(wpt)

            # ---- x / W_attn loads: W + q-block-0 columns first so B(0)
            # starts ~10us earlier; remaining column blocks stream behind
            x8t = [None] * NQB
            wa8 = qk8w.tile([128, 8 * 2 * HG], FP8, tag="wa8", name=f"wa8{_sfx}")
            nc.sync.dma_start(out=wa8[:], in_=wa8d[:])
            x8t[0] = x8p.tile([128, 8 * 512], FP8, tag="x8", name=f"x8_0{_sfx}")
            nc.sync.dma_start(
                out=x8t[0].rearrange("p (b c) -> p b c", c=512)[:, :, :],
                in_=x8d.rearrange("p (b c) -> p b c", c=T)[:, :, 0:512],
            )
            wav_t = wap.tile([128, 8 * HG], BF16, tag="wa", name=f"wav_t{_sfx}")
            nc.sync.dma_start(
                out=wav_t.rearrange("p (b c) -> p b c", c=HG)[:, :, :],
                in_=wav.rearrange("(b p) c -> p b c", p=128)[:, :, :],
            )
            xq_t = [None] * NQB
            xq_t[0] = xp.tile([128, 8 * 512], BF16, tag="xt", name=f"xq_0{_sfx}")
            nc.sync.dma_start(
                out=xq_t[0].rearrange("p (b c) -> p b c", c=512)[:, :, :],
                in_=xT.rearrange("(b p) t -> p b t", p=128)[:, :, 0:512],
            )

            qt8 = [qk.tile([64, 2 * T], FP8, tag="qt", name=f"qt8_{m}{_sfx}") for m in range(2)]
            kt8 = [qk.tile([64, 2 * T], FP8, tag="kt", name=f"kt8_{m}{_sfx}") for m in range(2)]
            yt = [ytp.tile([128, T], BF16, tag="yt", name=f"yt{m}{_sfx}") for m in range(2)]
            va_sb = [None] * KC

            # filler: deferred closures drained inside B(qb)'s exp-paced
            # window so the PE never idles waiting on the scalar engine
            filler = []

            def drain(n=None):
                k = len(filler) if n is None else min(n, len(filler))
                for _ in range(k):
                    filler.pop(0)()

            def emit_xq_loads(nb):
                def go(nb=nb):
                    x8t[nb] = x8p.tile([128, 8 * 512], FP8, tag="x8",
                                       name=f"x8_{nb}{_sfx}")
                    nc.sync.dma_start(
                        out=x8t[nb].rearrange("p (b c) -> p b c", c=512)[:, :, :],
                        in_=x8d.rearrange("p (b c) -> p b c", c=T)[:, :, nb * 512:(nb + 1) * 512],
                    )
                    xq_t[nb] = xp.tile([128, 8 * 512], BF16, tag="xt",
                                       name=f"xq_{nb}{_sfx}")
                    nc.sync.dma_start(
                        out=xq_t[nb].rearrange("p (b c) -> p b c", c=512)[:, :, :],
                        in_=xT.rearrange("(b p) t -> p b t", p=128)[:, :, nb * 512:(nb + 1) * 512],
                    )
                filler.append(go)

            def emit_sweep(nb):
                # m-split phases on single-bank psm tiles; per-phase fp8
                # convert + fold DMAs so B(nb) never waits on conversions
                st8 = {}

                def phase(m, half, nb=nb):
                    def go(m=m, half=half, nb=nb):
                        if half == 0:
                            ps_qm = psm.tile([128, 512], F32, tag="mm",
                                             name=f"ps_q{nb}_{m}{_sfx}")
                            ps_km = psm.tile([128, 512], F32, tag="mm",
                                             name=f"ps_k{nb}_{m}{_sfx}")
                            st8[m] = (ps_qm, ps_km)
                        ps_qm, ps_km = st8[m]
                        w3 = wa8.rearrange("p (b c) -> p b c", c=2 * HG)
                        r3 = x8t[nb].rearrange("p (b c) -> p b c", c=512)
                        for cp in range(2 * half, 2 * half + 2):
                            st, sp = (cp == 0), (cp == 3)
                            nc.tensor.matmul(
                                ps_qm[:],
                                w3[:, 2 * cp:2 * cp + 2, m * 128:(m + 1) * 128],
                                r3[:, 2 * cp:2 * cp + 2, :],
                                start=st, stop=sp, perf_mode=DR,
                            )
                            nc.tensor.matmul(
                                ps_km[:],
                                w3[:, 2 * cp:2 * cp + 2, HG + m * 128:HG + (m + 1) * 128],
                                r3[:, 2 * cp:2 * cp + 2, :],
                                start=st, stop=sp, perf_mode=DR,
                            )
                    return go

                def convfold(m, nb=nb):
                    def go(m=m, nb=nb):
                        c0, c1 = nb * 512, (nb + 1) * 512
                        ps_qm, ps_km = st8[m]
                        for src_ps, dst8, qk_tag in ((ps_qm, qt8, "q"), (ps_km, kt8, "k")):
                            f8 = f8p.tile([128, 512], FP8, tag="f8",
                                          name=f"f8_{qk_tag}{m}_{nb}{_sfx}")
                            nc.vector.tensor_scalar_mul(f8[:], src_ps[:], 1.0 / 16.0)
                            for j in range(2):
                                nc.sync.dma_start(
                                    out=dst8[m][:, j * T + c0:j * T + c1],
                                    in_=f8[j * 64:(j + 1) * 64, :],
                                )
                    return go

                for m in range(2):
                    filler.append(phase(m, 0))
                    filler.append(phase(m, 1))
                    filler.append(convfold(m))

            def emit_va(qb):
                def unit(tt):
                    def go(tt=tt):
                        ps = psm.tile([128, 512], F32, tag="mm", name=f"ps_v{tt}{_sfx}")
                        xv = xq_t[tt // 4].rearrange("p (b c) -> p b c", c=512)
                        wv3 = wav_t.rearrange("p (b c) -> p b c", c=HG)
                        for kc in range(8):
                            nc.tensor.matmul(
                                ps[:, 0:HG],
                                xv[:, kc, (tt % 4) * 128:(tt % 4 + 1) * 128],
                                wv3[:, kc, :],
                                start=(kc == 0), stop=(kc == 7),
                            )
                        va = vp.tile([128, HPC * 65], BF16, tag="va", name=f"va{tt}{_sfx}")
                        nc.vector.tensor_copy(
                            va.rearrange("p (h c) -> p h c", c=65)[:, :, 0:64],
                            ps[:, 0:HG].rearrange("p (h c) -> p h c", c=64)[:, :, :],
                        )
                        nc.gpsimd.memset(
                            va.rearrange("p (h c) -> p h c", c=65)[:, :, 64:65], 1.0)
                        va_sb[tt] = va
                    return go
                for tt in range(4 * qb, 4 * qb + 4):
                    filler.append(unit(tt))

            def emit_c(qb):
                last = qb == NQB - 1

                def unit(tt, nn):
                    def go(tt=tt, nn=nn):
                        ps = psm.tile([128, 512], F32, tag="mm",
                                      name=f"ps_c{tt}_{nn}{_sfx}")
                        for pr in range(2):
                            nc.tensor.matmul(
                                ps[:],
                                yt[pr][:, tt * 128:(tt + 1) * 128],
                                wp_sb[pr][:, nn * 512:(nn + 1) * 512],
                                start=(pr == 0), stop=(pr == 1),
                            )
                        osb = osb_sb[tt]
                        if last:
                            # scalar engine is idle once the exps are done
                            nc.scalar.activation(osb[:, nn * 512:(nn + 1) * 512],
                                                 ps[:], COPY)
                            nc.sync.dma_start(
                                out=out[tt * 128:(tt + 1) * 128, nn * 512:(nn + 1) * 512],
                                in_=osb[:, nn * 512:(nn + 1) * 512],
                            )
                        else:
                            nc.vector.tensor_copy(osb[:, nn * 512:(nn + 1) * 512], ps[:])
                            if nn == 1:
                                nc.sync.dma_start(
                                    out=out[tt * 128:(tt + 1) * 128, :], in_=osb[:])
                    return go
                osb_sb = {}
                for tt in range(4 * qb, 4 * qb + 4):
                    osb_sb[tt] = outp.tile([128, L], BF16, tag="osb",
                                           name=f"osb{tt}{_sfx}")
                    for nn in range(2):
                        filler.append(unit(tt, nn))

            quota = [0.0]
            per_step = [0.0]

            def pressure_drain():
                # drain uniformly across the window's remaining checkpoints
                quota[0] += per_step[0]
                while filler and quota[0] >= 1.0:
                    drain(1)
                    quota[0] -= 1.0

            def phase_start(qb, pr):
                nkc = 4 * qb + 4
                po = {}
                for hh in range(2):
                    po[hh] = pso.tile([65, 512], F32, tag="po",
                                      name=f"po{qb}_{pr}_{hh}{_sfx}")
                return {"qb": qb, "pr": pr, "nkc": nkc, "po": po,
                        "pts": [None] * nkc}

            def phase_qk(ctx, kc):
                qb, pr = ctx["qb"], ctx["pr"]
                j = kc - 4 * qb
                if j < 0:
                    a0, ncols = 0, 512
                else:
                    a0, ncols = 128 * j, 512 - 128 * j
                q0 = qb * 512 + a0
                ps_s = pss.tile([128, 1024], F32, tag="pss",
                                name=f"ps_s{qb}_{pr}_{kc}{_sfx}")
                k3 = kt8[pr].rearrange("p (j t) -> p j t", j=2)
                q3 = qt8[pr].rearrange("p (j t) -> p j t", j=2)
                for hh in range(2):
                    hb = hh * 32
                    nc.tensor.matmul(
                        ps_s[:, hh * 512 + a0:hh * 512 + 512],
                        k3[hb:hb + 32, :, kc * 128:(kc + 1) * 128],
                        q3[hb:hb + 32, :, q0:q0 + ncols],
                        start=True, stop=True,
                        perf_mode=DR,
                    )
                pt = ptp.tile([128, 1024], BF16, tag="pt",
                              name=f"pt{qb}_{pr}_{kc}{_sfx}")
                s3 = ps_s.rearrange("p (h c) -> p h c", c=512)[:, :, a0:512]
                d3 = pt.rearrange("p (h c) -> p h c", c=512)[:, :, a0:512]
                nc.scalar.activation(d3, s3, EXP, scale=float(SCALE))
                if j >= 0:
                    # zero the upper triangle of the diagonal block
                    for hh in range(2):
                        blk = pt[:, hh * 512 + a0:hh * 512 + a0 + 128]
                        nc.gpsimd.tensor_mul(blk, blk, tm_sb[:])
                ctx["pts"][kc] = (pt, a0, ncols)

            def phase_pv(ctx, kc):
                qb, pr, nkc = ctx["qb"], ctx["pr"], ctx["nkc"]
                while va_sb[kc] is None and filler:
                    drain(1)
                pt, a0, ncols = ctx["pts"][kc]
                for hh in range(2):
                    h = 2 * pr + hh
                    nc.tensor.matmul(
                        ctx["po"][hh][:, a0:512],
                        va_sb[kc][:, h * 65:(h + 1) * 65],
                        pt[:, hh * 512 + a0:hh * 512 + 512],
                        start=(kc == 0), stop=(kc == nkc - 1),
                        skip_group_check=(0 < kc < nkc - 1),
                    )
                ctx["pts"][kc] = None

            def phase_norm(ctx):
                # normalize: yt = po[0:64] * broadcast(1/po[64]); filler
                # covers the DVE reciprocal latency; broadcast lands in a
                # pss-pool tile (both heads share it)
                qb, pr, po = ctx["qb"], ctx["pr"], ctx["po"]
                # per-head pipeline: hh0's broadcast/multiply overlap hh1's
                # reciprocal, and po[0] releases one multiply earlier so the
                # next phase's PV stalls less on the pso pool
                recs = {}
                for hh in range(2):
                    rec = recp.tile([65, 512], BF16, tag="rec",
                                    name=f"rec{qb}_{pr}_{hh}{_sfx}")
                    with nc.allow_low_precision(reason="softmax denom reciprocal to bf16 for matmul broadcast"):
                        nc.vector.reciprocal(rec[64:65, :], po[hh][64:65, :])
                    recs[hh] = rec
                    if hh == 0:
                        ps_bp = pss.tile([128, 1024], F32, tag="pss",
                                         name=f"ps_bp{qb}_{pr}{_sfx}")
                        nc.tensor.matmul(ps_bp[:, 0:512],
                                         ones_sb[64:65, :], recs[0][64:65, :],
                                         start=True, stop=True)
                pressure_drain()
                bs = bsp.tile([128, 1024], BF16, tag="bs",
                              name=f"bs{qb}_{pr}{_sfx}")
                nc.vector.tensor_copy(bs[:, 0:512], ps_bp[:, 0:512])
                nc.tensor.matmul(ps_bp[:, 512:1024],
                                 ones_sb[64:65, :], recs[1][64:65, :],
                                 start=True, stop=True)
                nc.vector.tensor_mul(
                    yt[pr][0:64, qb * 512:(qb + 1) * 512],
                    po[0][0:64, :],
                    bs[0:64, 0:512],
                )
                nc.vector.tensor_copy(bs[:, 512:1024], ps_bp[:, 512:1024])
                nc.vector.tensor_mul(
                    yt[pr][64:128, qb * 512:(qb + 1) * 512],
                    po[1][0:64, :],
                    bs[64:128, 512:1024],
                )
                pressure_drain()

            # ---- sweep(0): m0 on psm tiles (freed by its conversion),
            # m1 on one pss tile, so B(0)'s first score tile allocates
            # into the remaining free pss slot immediately
            w3 = wa8.rearrange("p (b c) -> p b c", c=2 * HG)
            r3 = x8t[0].rearrange("p (b c) -> p b c", c=512)
            ps_q00 = psm.tile([128, 512], F32, tag="mm", name=f"ps_q00{_sfx}")
            ps_k00 = psm.tile([128, 512], F32, tag="mm", name=f"ps_k00{_sfx}")
            ps_m1 = pss.tile([128, 1024], F32, tag="pss", name=f"ps_m1{_sfx}")
            sw0 = {0: (ps_q00, ps_k00),
                   1: (ps_m1[:, 0:512], ps_m1[:, 512:1024])}
            for m in range(2):
                dq, dk = sw0[m]
                for cp in range(4):
                    st, sp = (cp == 0), (cp == 3)
                    nc.tensor.matmul(
                        dq, w3[:, 2 * cp:2 * cp + 2, m * 128:(m + 1) * 128],
                        r3[:, 2 * cp:2 * cp + 2, :],
                        start=st, stop=sp, perf_mode=DR,
                    )
                    nc.tensor.matmul(
                        dk, w3[:, 2 * cp:2 * cp + 2, HG + m * 128:HG + (m + 1) * 128],
                        r3[:, 2 * cp:2 * cp + 2, :],
                        start=st, stop=sp, perf_mode=DR,
                    )
                for src_ps, dst8, qk_tag in ((dq, qt8, "q"), (dk, kt8, "k")):
                    f8 = f8p.tile([128, 512], FP8, tag="f8",
                                  name=f"f8i_{qk_tag}{m}_0{_sfx}")
                    nc.vector.tensor_scalar_mul(f8[:], src_ps, 1.0 / 16.0)
                    for j in range(2):
                        nc.sync.dma_start(
                            out=dst8[m][:, j * T:j * T + 512],
                            in_=f8[j * 64:(j + 1) * 64, :],
                        )
            emit_va(0)
            phases = [(qb, pr) for qb in range(NQB) for pr in range(2)]
            ctx = None
            carried_qk0 = False
            for pi, (qb, pr) in enumerate(phases):
                if pr == 0:
                    if qb > 0:
                        emit_c(qb - 1)
                    if qb + 1 < NQB:
                        emit_xq_loads(qb + 1)
                        emit_sweep(qb + 1)
                        emit_va(qb + 1)
                    nkc = 4 * qb + 4
                    ckpts = 2 * (1 + 2 * (nkc - 1) + 3)
                    per_step[0] = len(filler) / ckpts if ckpts else 0.0
                    quota[0] = 0.0
                if ctx is None:
                    ctx = phase_start(qb, pr)
                    phase_qk(ctx, 0)
                nkc = ctx["nkc"]
                pressure_drain()
                for kc in range(1, nkc):
                    phase_qk(ctx, kc)
                    pressure_drain()
                    phase_pv(ctx, kc - 1)
                    pressure_drain()
                # lookahead: next phase's first scores + exp keep the scalar
                # engine busy through this phase's normalize
                nctx = None
                if pi + 1 < len(phases):
                    nqb, npr = phases[pi + 1]
                    nctx = phase_start(nqb, npr)
                    phase_qk(nctx, 0)
                phase_pv(ctx, nkc - 1)
                phase_norm(ctx)
                ctx = nctx
            drain()
            emit_c(NQB - 1)
            drain()

          for _rep in range(reps):
            _body(_rep)

    import os as _os
    if not _os.environ.get("KERNEL_SKIP_WAITFIX"):
        _fix_matmul_waits(nc)
    return nc


def _fix_matmul_waits(nc):
    """walrus caps sync-wait commands at one per hardware instruction.
    Tile can emit more. Two safe fixes, applied in order:
    1. drop waits on the instruction's own engine semaphore that are already
       guaranteed by in-order retirement of earlier same-stream instructions;
    2. for any instruction still holding >1 wait, insert same-engine NoOps
       immediately before it, each carrying one excess wait (the waits still
       all execute before the instruction dispatches).
    """
    import bass_rust
    import concourse.mybir as mybir

    SKIP = (mybir.InstEventSemaphore, mybir.InstCall,
            mybir.InstUnconditionalBranch)
    nop_id = [0]

    for f in nc.m.functions:
        for blk in f.blocks:
            insts = list(blk.instructions)
            out = []
            changed = False
            for i, inst in enumerate(insts):
                si = inst.sync_info
                eng = getattr(inst, "engine", None)
                if si is None or eng is None or isinstance(inst, SKIP):
                    out.append(inst)
                    continue
                waits = list(si.on_wait)
                kept = waits
                if len(kept) > 1:
                    for w in kept[:-1]:
                        nop = mybir.InstNoOp(name=f"I-waitnop-{nop_id[0]}")
                        nop_id[0] += 1
                        nop.engine = eng
                        nop.sync_info = bass_rust.SyncInfo(on_wait=[w], on_update=[])
                        out.append(nop)
                    kept = kept[-1:]
                if len(kept) != len(waits):
                    inst.sync_info = bass_rust.SyncInfo(
                        on_wait=kept, on_update=list(si.on_update))
                    changed = True
                out.append(inst)
            if changed or len(out) != len(insts):
                blk.instructions = out


def make_in_maps(x, W_attn, W_proj):
    x = np.ascontiguousarray(np.asarray(x, dtype=np.float32))
    W_attn = np.ascontiguousarray(np.asarray(W_attn, dtype=np.float32))
    W_proj = np.ascontiguousarray(np.asarray(W_proj, dtype=np.float32))
    # [k, q] layout: invalid where q < k gets -3e30 (becomes exp -> 0).
    trimaskb = np.triu(np.ones((128, 128), np.float32))
    in_maps = []
    for c in range(N_CORES):
        b, hg = c // 4, c % 4
        cs = slice(hg * HG, (hg + 1) * HG)
        wqk = np.concatenate(
            [W_attn[:, 0 * L:][:, cs], W_attn[:, 1 * L:][:, cs]], axis=1)
        wv = W_attn[:, 2 * L:][:, cs]
        import ml_dtypes
        bf16 = ml_dtypes.bfloat16
        fp8 = ml_dtypes.float8_e4m3
        xTb = x[b].T                                     # [L, T]
        # chunk-folded fp8 copies for DoubleRow: [128, 8 blocks, cols]
        x8 = xTb.reshape(8, 128, T).transpose(1, 0, 2).reshape(128, 8 * T)
        wa8 = 16.0 * wqk.reshape(8, 128, 2 * HG).transpose(1, 0, 2)
        # permute each 128-col slice: d=(hh,j,p) -> pi=(j,hh,p) so the fold
        # DMA reads contiguous 64-partition blocks per j
        d = np.arange(128)
        pi = (d % 64) // 32 * 64 + d // 64 * 32 + d % 32
        inv = np.empty(128, np.int64); inv[pi] = d
        wa8 = wa8.reshape(128, 8, 4, 128)[:, :, :, inv].reshape(128, 8 * 2 * HG)
        in_maps.append({
            "xT": np.ascontiguousarray(xTb.astype(bf16)),
            "x8": np.ascontiguousarray(x8.astype(fp8)),
            "wav": np.ascontiguousarray(wv.astype(bf16)),
            "wa8": np.ascontiguousarray(wa8.astype(fp8)),
            "wp": np.ascontiguousarray(W_proj[cs, :].astype(bf16)),
            "trimaskb": trimaskb,
        })
    return in_maps


_NC_CACHE = None


def kernel(x, W_attn, W_proj, **run_kwargs):
    global _NC_CACHE
    from concourse.bass_utils import run_bass_kernel_spmd

    if _NC_CACHE is None:
        _NC_CACHE = build_nc()
    nc = _NC_CACHE
    in_maps = make_in_maps(x, W_attn, W_proj)
    res = run_bass_kernel_spmd(nc, in_maps, list(range(N_CORES)), **run_kwargs)
    results = res.results if hasattr(res, "results") else res
    out = np.zeros((B, T, L), np.float32)
    for c in range(N_CORES):
        out[c // 4] += np.asarray(results[c]["out"], dtype=np.float32)
    if run_kwargs:
        kernel.last_results = res
    return out
